# revision 14
# baseline (speedup 1.0000x reference)
"""Trainium2 Bass kernel: causal multi-head attention block (B=2,S=2048,H=2048,NH=16,HD=128).

Sharding: 8 cores = DP over batch (2) x TP over head-groups (4 groups of 4 heads).
Each core computes q/k/v projections for its 4 heads, RoPE, causal softmax
attention, and a partial output projection; the host sums the 4 partials per
batch and adds bo.

v4 structure:
  All DMAs are few and large: every device input is pre-rearranged on the host
  into partition-major [128, ...] layout so each transfer is per-partition
  contiguous, because Tile serializes same-lane HWDGE DMAs (each waits the
  previous completion, ~2us dead time per DMA). Small constants are packed
  into single tensors. y is staged through [128, 2048] bf16 tiles (one DMA per
  128-row stripe); the host sums partials in f32.
  Phase A (per 512-col s-chunk): Q, K, V projection matmuls back-to-back on
  the PE sharing one x tile; evacuations (ACT bias + rope matmul + ACT
  rot-copy + three 2-byte DVE ops with fp16 cos/sin) run under the following
  sections.
  Phase B (per 512-row q-chunk, per head): transposed score pairs ST[k,q] into
  2-bank PSUM tiles, additive bf16 causal mask, one [128,1024] exp per pair,
  PV + denominator on the PE with diagonal column narrowing; the previous
  chunk's output projection drips between pairs as PE filler.
"""

import math
import os
import sys

import numpy as np

for _p in ("/opt/trn_rl_repo",):
    if _p not in sys.path and os.path.isdir(_p):
        sys.path.insert(0, _p)

import ml_dtypes

import concourse.bass as bass
import concourse.mybir as mybir
import concourse.tile as tile
from concourse import bacc

B, S, H, NH, HD = 2, 2048, 2048, 16, 128
NCORES = 8
HG = 4            # head-groups (TP degree)
HPG = NH // HG    # heads per group = 4
DLOC = HPG * HD   # local d width = 512
FT = H // 128     # 16 f-tiles
SJ = S // 512     # 4 s/q tiles of 512
KT128 = S // 128  # 16 k-tiles of 128
NEG = -1e30

F32 = mybir.dt.float32
F16 = mybir.dt.float16
BF16 = mybir.dt.bfloat16
NPBF16 = ml_dtypes.bfloat16

# wq / x first-chunk split points (f-tiles) for early PE start
_SPLITS = [(0, 2), (2, 6), (6, 11), (11, 16)]


def build_program(mode: str, hwdge_sems: int) -> bass.Bass:
    """mode in {'causal', 'full', 'bias'}"""
    import concourse.tile_sem_assignment as tsa

    tsa.NUM_HWDGE_SEMS = hwdge_sems
    tsa.NUM_SWDGE_GLOBAL_SEMS = 1
    nc = bacc.Bacc()
    # all inputs are host-side pre-rearranged to partition-major layouts
    xd = nc.dram_tensor("xd", [128, SJ, FT, 512], BF16, kind="ExternalInput")
    wqd = nc.dram_tensor("wqd", [128, FT, DLOC], BF16, kind="ExternalInput")
    wkd = nc.dram_tensor("wkd", [128, FT, DLOC], BF16, kind="ExternalInput")
    wvd = nc.dram_tensor("wvd", [128, FT, DLOC], BF16, kind="ExternalInput")
    wod = nc.dram_tensor("wod", [128, HPG, H], BF16, kind="ExternalInput")
    # cpack: bq[4] | bk[4] | bv[512] (f32)
    cpack = nc.dram_tensor("cpack", [128, 520], F32, kind="ExternalInput")
    # cs: cos[2048] | sin[2048] (f16)
    cs = nc.dram_tensor("cs", [128, 2 * S], F16, kind="ExternalInput")
    rmat = nc.dram_tensor("rmat", [HD, HD], BF16, kind="ExternalInput")
    if mode == "causal":
        dmask = nc.dram_tensor("dmask", [128, 2048], BF16, kind="ExternalInput")
    elif mode == "bias":
        fbias = nc.dram_tensor("fbias", [S, S], F32, kind="ExternalInput")
    y = nc.dram_tensor("y", [SJ * 4, 128, H], BF16, kind="ExternalOutput")

    with tile.TileContext(nc) as tc:
        with (
            tc.tile_pool(name="qt", bufs=HPG * SJ) as qt_pool,
            tc.tile_pool(name="kt", bufs=HPG * SJ) as kt_pool,
            tc.tile_pool(name="vt", bufs=KT128) as vt_pool,
            tc.tile_pool(name="consts", bufs=1) as consts,
        ):
            QT = {}
            KT = {}
            VT = {}

            ones_sb = consts.tile([128, 1], F16, tag="ones")
            nc.gpsimd.memset(ones_sb[:], 1.0)
            onesr_sb = consts.tile([1, 128], F16, tag="onesr")
            nc.gpsimd.memset(onesr_sb[:], 1.0)
            wo_sb = consts.tile([128, HPG, H], BF16, tag="wo")
            cp_sb = consts.tile([128, 520], F32, tag="cp")
            bq_sb = cp_sb[:, 0:4]
            bk_sb = cp_sb[:, 4:8]
            bv_sb = cp_sb[:, 8:520]
            rmat_sb = consts.tile([HD, HD], BF16, tag="rmat")
            dm_sb = None
            if mode == "causal":
                dm_sb = consts.tile([128, 2048], BF16, tag="dm")

            # ============ Phase A: fused Q/K/V projections + RoPE ============
            with (
                tc.tile_pool(name="pa", bufs=8, space="PSUM") as pa,
                tc.tile_pool(name="wqk", bufs=1) as wqk_pool,
                tc.tile_pool(name="csn", bufs=1) as csn_pool,
                tc.tile_pool(name="xin", bufs=2) as xin_pool,
                tc.tile_pool(name="rtmp", bufs=3) as rtmp_pool,
                tc.tile_pool(name="rot", bufs=3) as rot_pool,
            ):
                wq_sb = wqk_pool.tile([128, FT, DLOC], BF16, tag="wq")
                wk_sb = wqk_pool.tile([128, FT, DLOC], BF16, tag="wk")
                wv_sb = wqk_pool.tile([128, FT, DLOC], BF16, tag="wv")
                cs_sb = csn_pool.tile([128, 2 * S], F16, tag="cs")

                xts = [
                    xin_pool.tile([128, FT, 512], BF16, tag="xt", name="xt")
                    for _ in range(SJ)
                ]

                def bias_evac(which, h, ps, bias_sb, store, sj):
                    # ACT: psum -> bf16 tile with per-head bias
                    pool = qt_pool if which == "q" else kt_pool
                    t = pool.tile([128, 512], BF16, tag="t", name="qkt")
                    nc.scalar.activation(
                        t[:], ps[:],
                        mybir.ActivationFunctionType.Identity,
                        bias=bias_sb[:, h : h + 1],
                    )
                    store[(h, sj)] = t
                    return t

                def rope_mm(t):
                    rp = pa.tile([128, 512], F32, tag="ps", name="rp")
                    nc.tensor.matmul(rp[:], rmat_sb[:], t[:], start=True, stop=True)
                    return rp

                def rope_fin(t, rp, sj):
                    rot = rot_pool.tile([128, 512], BF16, tag="rot", name="rot")
                    nc.scalar.activation(
                        rot[:], rp[:], mybir.ActivationFunctionType.Identity
                    )
                    tmp = rtmp_pool.tile([128, 512], BF16, tag="tmp", name="tmp")
                    sss = cs_sb[:, S + sj * 512 : S + (sj + 1) * 512]
                    css = cs_sb[:, sj * 512 : (sj + 1) * 512]
                    nc.vector.tensor_mul(tmp[:], rot[:], sss)
                    nc.vector.tensor_mul(t[:], t[:], css)
                    nc.vector.tensor_add(t[:], t[:], tmp[:])

                with nc.named_scope("phaseA"):
                    for sj in range(SJ):
                        xt = xts[sj]
                        if sj == 0:
                            for f0, f1 in _SPLITS:
                                nc.sync.dma_start(wq_sb[:, f0:f1, :], wqd[:, f0:f1, :])
                                nc.sync.dma_start(xt[:, f0:f1, :], xd[:, 0, f0:f1, :])
                            nc.sync.dma_start(wk_sb[:], wkd[:])
                            nc.sync.dma_start(rmat_sb[:], rmat[:])
                            nc.sync.dma_start(cp_sb[:], cpack[:])
                            nc.sync.dma_start(cs_sb[:], cs[:])
                            nc.sync.dma_start(wv_sb[:], wvd[:])
                            nc.sync.dma_start(xts[1][:], xd[:, 1])
                            nc.sync.dma_start(wo_sb[:], wod[:])
                            if mode == "causal":
                                nc.sync.dma_start(dm_sb[:], dmask[:])
                        elif sj + 1 < SJ:
                            nc.sync.dma_start(xts[sj + 1][:], xd[:, sj + 1])

                        qp = [pa.tile([128, 512], F32, tag="ps", name="ps") for _ in range(HPG)]
                        for ft in range(FT):
                            for h in range(HPG):
                                nc.tensor.matmul(
                                    qp[h][:],
                                    wq_sb[:, ft, h * 128 : (h + 1) * 128],
                                    xt[:, ft, :],
                                    start=(ft == 0),
                                    stop=(ft == FT - 1),
                                )
                        # bias ACTs run under the K section
                        tqs = [bias_evac("q", h, qp[h], bq_sb, QT, sj) for h in range(HPG)]
                        kp = [pa.tile([128, 512], F32, tag="ps", name="ps") for _ in range(HPG)]
                        for ft in range(FT):
                            for h in range(HPG):
                                nc.tensor.matmul(
                                    kp[h][:],
                                    wk_sb[:, ft, h * 128 : (h + 1) * 128],
                                    xt[:, ft, :],
                                    start=(ft == 0),
                                    stop=(ft == FT - 1),
                                )
                        tks = [bias_evac("k", h, kp[h], bk_sb, KT, sj) for h in range(HPG)]
                        # rope matmuls interleave into the V section (2 per V
                        # group) so the rot/DVE chains finish with the chunk
                        # instead of trailing into the next one
                        tall = tqs + tks
                        rps = []
                        for ss in range(4):
                            vp = pa.tile([128, 512], F32, tag="ps", name="vp")
                            for ft in range(FT):
                                nc.tensor.matmul(
                                    vp[:],
                                    xt[:, ft, ss * 128 : (ss + 1) * 128],
                                    wv_sb[:, ft, :],
                                    start=(ft == 0),
                                    stop=(ft == FT - 1),
                                )
                            for t in tall[2 * ss : 2 * ss + 2]:
                                rps.append(rope_mm(t))
                            v = vt_pool.tile([128, DLOC], BF16, tag="v", name="v")
                            nc.vector.tensor_add(v[:], vp[:], bv_sb[:])
                            VT[4 * sj + ss] = v
                        for t, rp in zip(tall, rps):
                            rope_fin(t, rp, sj)

            # ============ Phase B: attention + output projection ============
            with (
                tc.tile_pool(name="pst", bufs=2, space="PSUM") as psum_st,
                tc.tile_pool(name="ppv", bufs=2, space="PSUM") as psum_pv,
                tc.tile_pool(name="pms", bufs=2, space="PSUM") as psum_ms,
                tc.tile_pool(name="ex", bufs=4) as exp_pool,
                tc.tile_pool(name="ot", bufs=2 * HPG) as ot_pool,
                tc.tile_pool(name="rc", bufs=6) as rc_pool,
                tc.tile_pool(name="ysb", bufs=2) as y_pool,
                tc.tile_pool(name="fb", bufs=3) as fb_pool,
            ):
                pending = []   # out-proj filler closures
                evac_flip = [0]

                def pop_filler(slots_left):
                    n = len(pending)
                    if not n:
                        return
                    k = max(1, -(-n // max(1, slots_left)))
                    for _ in range(min(k, n)):
                        pending.pop(0)()

                with nc.named_scope("phaseB"):
                    # largest q-chunk first: keeps the PE dense right after
                    # phase A (no ACT-bound warmup window), fillers backfill
                    # the small chunks
                    for qj in (3, 2, 1, 0):
                        OT = {}
                        PV = {}
                        RCH = {}

                        def _normalize(i):
                            rcb_ps = psum_ms.tile([128, 512], F32, tag="ms", name="rcb_ps")
                            nc.tensor.matmul(
                                rcb_ps[:], onesr_sb[:], RCH[i][:], start=True, stop=True
                            )
                            rcb = rc_pool.tile([128, 512], F16, tag="rcb", name="rcb")
                            nc.vector.tensor_copy(rcb[:], rcb_ps[:])
                            ot = ot_pool.tile([128, 512], BF16, tag="ot", name="ot")
                            nc.vector.tensor_mul(ot[:], PV[i][:], rcb[:])
                            OT[i] = ot

                        kmax = 4 * qj + 4 if mode == "causal" else KT128
                        P = kmax // 2
                        slots = [4 * P]

                        hstate = {}

                        def qk_exp(h, p):
                            st = psum_st.tile([128, 1024], F32, tag="st", name="st")
                            for half in (0, 1):
                                kj = 2 * p + half
                                a = kj - 4 * qj
                                off = 128 * a if (mode == "causal" and a > 0) else 0
                                nc.tensor.matmul(
                                    st[:, half * 512 + off : (half + 1) * 512],
                                    KT[(h, kj // 4)][:, (kj % 4) * 128 : (kj % 4 + 1) * 128],
                                    QT[(h, qj)][:, off:],
                                    start=True,
                                    stop=True,
                                )
                            if mode == "causal" and p >= 2 * qj:
                                variant = p - 2 * qj  # 0 or 1
                                for half in (0, 1):
                                    a = 2 * variant + half
                                    w = min(512, 128 * (a + 1))
                                    c0 = half * 512
                                    nc.vector.tensor_add(
                                        st[:, c0 : c0 + w],
                                        st[:, c0 : c0 + w],
                                        dm_sb[:, variant * 1024 + c0 : variant * 1024 + c0 + w],
                                    )
                            elif mode == "bias":
                                fb = fb_pool.tile([128, 1024], F32, tag="fb", name="fb")
                                for half in (0, 1):
                                    kj = 2 * p + half
                                    nc.sync.dma_start(
                                        fb[:, half * 512 : (half + 1) * 512],
                                        fbias[
                                            kj * 128 : (kj + 1) * 128,
                                            qj * 512 : (qj + 1) * 512,
                                        ],
                                    )
                                nc.vector.tensor_add(st[:], st[:], fb[:])
                            e2 = exp_pool.tile([128, 1024], BF16, tag="e", name="e")
                            nc.scalar.activation(
                                e2[:], st[:], mybir.ActivationFunctionType.Exp
                            )
                            return e2

                        def do_pv(h, p_, e2p):
                            if p_ == 0:
                                hstate[h] = (
                                    psum_pv.tile([128, 512], F32, tag="pv", name="pv"),
                                    psum_ms.tile([1, 512], F32, tag="ms", name="dnp"),
                                )
                            pv, dnp = hstate[h]
                            for half in (0, 1):
                                kj = 2 * p_ + half
                                a = kj - 4 * qj
                                off = 128 * a if (mode == "causal" and a > 0) else 0
                                c0 = half * 512 + off
                                c1 = (half + 1) * 512
                                nc.tensor.matmul(
                                    pv[:, off:],
                                    VT[kj][:, h * 128 : (h + 1) * 128],
                                    e2p[:, c0:c1],
                                    start=(kj == 0),
                                    stop=(kj == kmax - 1),
                                )
                                nc.tensor.matmul(
                                    dnp[:, off:],
                                    ones_sb[:],
                                    e2p[:, c0:c1],
                                    start=(kj == 0),
                                    stop=(kj == kmax - 1),
                                )

                        def finish_head(h):
                            pv, dnp = hstate[h]
                            rcf = rc_pool.tile([1, 512], F32, tag="rcf", name="rcf")
                            nc.vector.reciprocal_approx_fast(rcf[:], dnp[:])
                            rch = rc_pool.tile([1, 512], F16, tag="rch", name="rch")
                            nc.vector.tensor_copy(rch[:], rcf[:])
                            PV[h] = pv
                            RCH[h] = rch
                            if h > 0:
                                _normalize(h - 1)

                        # flat software pipeline across heads: exp(h,p) issued
                        # one slot ahead of its PV so the PE never drains at
                        # head boundaries
                        prev = None
                        for it in [(h, p) for h in range(HPG) for p in range(P)]:
                            e2 = qk_exp(*it)
                            if prev is not None:
                                (ph, pp), pe2 = prev
                                do_pv(ph, pp, pe2)
                                pop_filler(slots[0])
                                slots[0] -= 1
                                if pp == P - 1:
                                    finish_head(ph)
                            prev = (it, e2)
                        (ph, pp), pe2 = prev
                        do_pv(ph, pp, pe2)
                        pop_filler(slots[0])
                        slots[0] -= 1
                        finish_head(HPG - 1)
                        _normalize(HPG - 1)

                        OTs = [OT[dt] for dt in range(HPG)]

                        def mk_row(ss, OTl, qjl):
                            # one 128-row y stripe: 4 oj quadrants into one
                            # [128, 2048] staging tile, then a single DMA
                            ysb_box = [None]

                            def mk(oj):
                                def go():
                                    if ysb_box[0] is None:
                                        ysb_box[0] = y_pool.tile(
                                            [128, 2048], BF16, tag="y", name="y"
                                        )
                                    ysb = ysb_box[0]
                                    yp = psum_ms.tile([128, 512], F32, tag="ms", name="yp")
                                    for dt in range(HPG):
                                        nc.tensor.matmul(
                                            yp[:],
                                            OTl[dt][:, ss * 128 : (ss + 1) * 128],
                                            wo_sb[:, dt, oj * 512 : (oj + 1) * 512],
                                            start=(dt == 0),
                                            stop=(dt == HPG - 1),
                                        )
                                    sl = ysb[:, oj * 512 : (oj + 1) * 512]
                                    if evac_flip[0] % 2 == 0:
                                        nc.vector.tensor_copy(sl, yp[:])
                                    else:
                                        nc.scalar.activation(
                                            sl, yp[:],
                                            mybir.ActivationFunctionType.Identity,
                                        )
                                    evac_flip[0] += 1
                                    if oj == 3:
                                        nc.sync.dma_start(y[qjl * 4 + ss], ysb[:])

                                return go

                            return [mk(oj) for oj in range(4)]

                        for ss in range(4):
                            pending.extend(mk_row(ss, OTs, qj))
                    while pending:
                        pending.pop(0)()
    nc.compile()
    return nc


_PROGRAM_CACHE = {}


def _get_program(mode):
    if mode not in _PROGRAM_CACHE:
        try:
            _PROGRAM_CACHE[mode] = build_program(mode, 2)
        except Exception as e:
            print(f"build with 2 hwdge sems failed ({type(e).__name__}); retrying with 1")
            _PROGRAM_CACHE[mode] = build_program(mode, 1)
    return _PROGRAM_CACHE[mode]


def _detect_mode(attn_mask):
    m = np.asarray(attn_mask).reshape(S, S)
    if (m == np.tril(np.ones((S, S), m.dtype))).all():
        return "causal"
    if (m != 0).all():
        return "full"
    return "bias"


def _rot_matrix():
    r = np.zeros((HD, HD), np.float32)
    for dp in range(HD):
        if dp % 2 == 0:
            r[dp + 1, dp] = -1.0
        else:
            r[dp - 1, dp] = 1.0
    return r


def _diag_mask2():
    # [128 rows(k), 2 variants * 1024 cols]: variant v half hf covers k-tile
    # a = 2v+hf of the diagonal group; allowed iff col >= 128*a + row.
    out = np.zeros((2, 128, 1024), np.float32)
    r = np.arange(128)[:, None]
    c = np.arange(512)[None, :]
    for v in range(2):
        for hf in range(2):
            a = 2 * v + hf
            out[v, :, hf * 512 : (hf + 1) * 512] = np.where(
                c >= 128 * a + r, 0.0, NEG
            )
    return np.ascontiguousarray(out.transpose(1, 0, 2).reshape(128, 2048)).astype(
        NPBF16
    )


def _bf16(a):
    return np.ascontiguousarray(a).astype(NPBF16)


def _pm3(mat, inner):
    """[ (g p), d ] -> partition-major [128, g, d] with g groups."""
    g = mat.shape[0] // 128
    return np.ascontiguousarray(mat.reshape(g, 128, inner).transpose(1, 0, 2))


def kernel(**inputs) -> np.ndarray:
    from concourse.bass_utils import run_bass_kernel_spmd

    x = np.asarray(inputs["x"], np.float32)
    fcos = np.asarray(inputs["fcos"], np.float32)
    fsin = np.asarray(inputs["fsin"], np.float32)
    Wq, bq = np.asarray(inputs["Wq"], np.float32), np.asarray(inputs["bq"], np.float32)
    Wk, bk = np.asarray(inputs["Wk"], np.float32), np.asarray(inputs["bk"], np.float32)
    Wv, bv = np.asarray(inputs["Wv"], np.float32), np.asarray(inputs["bv"], np.float32)
    Wo, bo = np.asarray(inputs["Wo"], np.float32), np.asarray(inputs["bo"], np.float32)
    attn_mask = inputs["attn_mask"]

    mode = _detect_mode(attn_mask)
    nc = _get_program(mode)

    sc = 1.0 / math.sqrt(HD)
    csb = np.concatenate([fcos.T, fsin.T], axis=1).astype(np.float16)
    shared = {
        "cs": np.ascontiguousarray(csb),
        "rmat": _rot_matrix().astype(NPBF16),
    }
    if mode == "causal":
        shared["dmask"] = _diag_mask2()
    elif mode == "bias":
        m = np.asarray(attn_mask).reshape(S, S)
        shared["fbias"] = np.ascontiguousarray(
            np.where(m.T == 0, NEG, 0.0).astype(np.float32)
        )

    in_maps = []
    for c in range(NCORES):
        b, hg = divmod(c, HG)
        rows = slice(DLOC * hg, DLOC * (hg + 1))
        xT = _bf16(x[b].T)  # [H, S]
        xdev = np.ascontiguousarray(
            xT.reshape(FT, 128, SJ, 512).transpose(1, 2, 0, 3)
        )
        cp = np.concatenate(
            [
                (bq[rows] * sc).reshape(HPG, 128).T,
                bk[rows].reshape(HPG, 128).T,
                np.broadcast_to(bv[rows].reshape(1, DLOC), (128, DLOC)),
            ],
            axis=1,
        ).astype(np.float32)
        in_maps.append(
            {
                "xd": xdev,
                "wqd": _pm3(_bf16((Wq[rows] * sc).T), DLOC),
                "wkd": _pm3(_bf16(Wk[rows].T), DLOC),
                "wvd": _pm3(_bf16(Wv[rows].T), DLOC),
                "wod": _pm3(_bf16(Wo[:, rows].T), H),
                "cpack": np.ascontiguousarray(cp),
                **shared,
            }
        )

    trace = bool(int(os.environ.get("KERNEL_TRACE", "0")))
    res = run_bass_kernel_spmd(nc, in_maps, list(range(NCORES)), trace=trace)
    if trace and res.exec_time_ns is not None:
        print(f"HW exec time: {res.exec_time_ns} ns")
        if res.per_core_scope_times:
            for scope, cores in res.per_core_scope_times.items():
                if cores:
                    vals = list(cores.values())
                    print(f"  scope {scope}: max={max(vals)/1e3:.1f}us")
        globals()["LAST_EXEC_NS"] = res.exec_time_ns
        globals()["LAST_RESULTS"] = res

    out = np.zeros((B, S, H), np.float32)
    for c in range(NCORES):
        yv = res.results[c]["y"].astype(np.float32)  # [16, 128, H]
        out[c // HG] += yv.reshape(S, H)
    out += bo
    return out
